# revision 9
# baseline (speedup 1.0000x reference)
"""BiGRU Trainium2 kernel: B=64, T=512, D=256, U=512, 8 NeuronCores.

Sharding: 8 cores = 2 directions x 4 batch-quarters (B_local=16).
The GRU recurrence is latency-bound (T sequential steps); each core runs one
chain for (direction, batch-quarter) with a packed layout:
  partition p = 32*g + b   (g = U-block 0..3, b = local batch 0..15)
  column   c = offset within U-block (0..127);  u = 128*g + c

Gate matmuls are column-tiled across the 4 PE column groups (concurrent on
HW).  The state is kept TRANSPOSED only: hT32 [c, (g,b)] in fp32 (exact
accumulation; also the DMA'd output) plus a bf16 copy hT16 for the matmul
stationary operand.  Update:  hT_new = a + uT*hhT,  a = hT32 - uT*hT32
(a computed off the critical chain).  Filler matmuls into a junk PSUM bank
keep the PE HAM clock gate open (2.4 GHz) through the serial-chain gaps.
"""

import sys
import os

for _p in ("/opt/trn_rl_repo",):
    if os.path.isdir(_p) and _p not in sys.path:
        sys.path.insert(0, _p)

import numpy as np
from contextlib import ExitStack

import concourse.bass as bass
import concourse.bacc as bacc
import concourse.tile as tile
from concourse import mybir
from concourse.bass_utils import run_bass_kernel_spmd

try:
    from ml_dtypes import bfloat16
except ImportError:  # pragma: no cover
    import jax.numpy as _jnp

    bfloat16 = _jnp.bfloat16

B, T, D, U = 64, 512, 256, 512
NCORES = 8
BL = B // 4  # 16 local batch per core (4 batch quarters x 2 directions)
NG = 4  # U blocks of 128
KC_H = 4  # contraction chunks over U (512/128)
KC_X = 2  # contraction chunks over D (256/128)

F32 = mybir.dt.float32
BF16 = mybir.dt.bfloat16

OUT_BLOCK = 8  # steps per output DMA flush

# PE warm-keeping filler counts (matmuls into a junk PSUM bank placed in the
# PE stream where the serial chain would otherwise leave the array idle long
# enough for the HAM clock gate to re-throttle to 1.2 GHz).
FILL_SIG = 10   # during sigmoid(r) latency
FILL_TT = 4     # during tT = rT*hT
FILL_TANH = 14  # during tanh latency
FILL_TAIL = 10  # during b/hT_new tail
# ACT pacing: keep the Scalar queue busy from tanh(t-1) until sigmoid_r(t)'s
# input is ready -- an ACT op whose wait is satisfied at dispatch starts in
# ~40ns, one that sleeps at the queue head pays ~700ns wakeup.
PACE_SIG_N = 2432


def build_program(t_steps=T, with_bias=False, fills=None):
    """Builds the SPMD Bass program (identical for all cores)."""
    f_sig, f_tt, f_tanh, f_tail = fills or (FILL_SIG, FILL_TT, FILL_TANH, FILL_TAIL)
    # Bacc (not plain Bass): its compile pipeline splits multi-sem waits into
    # EventSemaphore instructions — TRN2 instructions hold at most one wait.
    nc = bacc.Bacc(None, target_bir_lowering=False)

    xT = nc.dram_tensor("xT", [128, KC_X, t_steps, BL], BF16, kind="ExternalInput")
    wh_ur = nc.dram_tensor("wh_ur", [128, KC_H, NG, 256], BF16, kind="ExternalInput")
    wh_hh = nc.dram_tensor("wh_hh", [128, KC_H, NG, 128], BF16, kind="ExternalInput")
    wx_all = nc.dram_tensor("wx_all", [128, KC_X, NG, 384], BF16, kind="ExternalInput")
    ident = nc.dram_tensor("ident", [128, 128], F32, kind="ExternalInput")
    ident16 = nc.dram_tensor("ident16", [128, 128], BF16, kind="ExternalInput")
    zrow = nc.dram_tensor("zrow", [1, 384], BF16, kind="ExternalInput")
    ones_row = nc.dram_tensor("ones_row", [1, 128], BF16, kind="ExternalInput")
    if with_bias:
        bias_all = nc.dram_tensor("bias_all", [1, NG, 384], BF16, kind="ExternalInput")
    out = nc.dram_tensor("out_packed", [128, t_steps, 128], F32, kind="ExternalOutput")

    with tile.TileContext(nc) as tc, ExitStack() as ctx:
        singles = ctx.enter_context(tc.tile_pool(name="singles", bufs=1))
        hT_pool = ctx.enter_context(tc.tile_pool(name="hT", bufs=3))
        gates = ctx.enter_context(tc.tile_pool(name="gates", bufs=3))
        stage_pool = ctx.enter_context(tc.tile_pool(name="stage", bufs=2))
        # bufs=1: the cur/next allocation pattern already makes 2 live slots
        # per pool (PSUM slots are bank-rounded; 8 banks total).
        ps_ur = ctx.enter_context(tc.tile_pool(name="ps_ur", bufs=1, space="PSUM"))
        ps_h = ctx.enter_context(tc.tile_pool(name="ps_h", bufs=1, space="PSUM"))
        ps_rT = ctx.enter_context(tc.tile_pool(name="ps_rT", bufs=1, space="PSUM"))
        ps_uT = ctx.enter_context(tc.tile_pool(name="ps_uT", bufs=1, space="PSUM"))
        ps_hhT = ctx.enter_context(tc.tile_pool(name="ps_hhT", bufs=1, space="PSUM"))
        ps_junk = ctx.enter_context(tc.tile_pool(name="ps_junk", bufs=1, space="PSUM"))

        # --- resident inputs ---
        xT_sb = singles.tile([128, KC_X, t_steps, BL], BF16)
        wh_ur_sb = singles.tile([128, KC_H, NG, 256], BF16)
        wh_hh_sb = singles.tile([128, KC_H, NG, 128], BF16)
        wx_sb = singles.tile([128, KC_X, NG, 384], BF16)
        ident_sb = singles.tile([128, 128], F32)
        ident16_sb = singles.tile([128, 128], BF16)
        nc.sync.dma_start(out=ident16_sb[:], in_=ident16[:])
        nc.sync.dma_start(out=xT_sb[:], in_=xT[:])
        nc.sync.dma_start(out=wh_ur_sb[:], in_=wh_ur[:])
        nc.sync.dma_start(out=wh_hh_sb[:], in_=wh_hh[:])
        nc.sync.dma_start(out=wx_sb[:], in_=wx_all[:])
        nc.sync.dma_start(out=ident_sb[:], in_=ident[:])
        zrow_sb = singles.tile([1, 384], BF16)
        ones_sb = singles.tile([1, 128], BF16)
        nc.sync.dma_start(out=zrow_sb[:], in_=zrow[:])
        nc.sync.dma_start(out=ones_sb[:], in_=ones_row[:])
        if with_bias:
            bias_sb = singles.tile([1, NG, 384], BF16)
            nc.sync.dma_start(out=bias_sb[:], in_=bias_all[:])

        # --- initial state (h = 0) ---
        h0_16 = singles.tile([128, 128], BF16)
        h0_32 = singles.tile([128, 128], F32)
        nc.vector.memset(h0_16[:], 0.0)
        nc.vector.memset(h0_32[:], 0.0)
        pace_src = singles.tile([128, 2560], BF16)
        pace_dst = singles.tile([128, 2560], BF16)
        nc.vector.memset(pace_src[:], 0.0)

        junk = ps_junk.tile([128, 128], F32, tag="junk")

        def emit_fill(n):
            for _ in range(n):
                nc.tensor.matmul(
                    junk[0:16, :],
                    lhsT=wh_ur_sb[:, 0, 0, 0:16],
                    rhs=wh_ur_sb[:, 0, 0, 0:128],
                    start=True,
                    stop=True,
                    tile_position=(0, 0),
                    skip_group_check=True,
                )

        # Pre-consume idents on PE: transpose-mode matmuls lower to the LDW
        # struct which holds only ONE sync wait; without this the first real
        # transpose would need both its data wait and the ident-DMA wait.
        warm_rT = ps_rT.tile([128, 128], BF16, tag="rT")
        nc.tensor.transpose(warm_rT[:], ident16_sb[:], ident16_sb[:])
        warm_uT = ps_uT.tile([128, 128], F32, tag="uT")
        nc.tensor.transpose(warm_uT[:], ident_sb[:], ident_sb[:])

        # HAM warm-up: ~5us of back-to-back matmuls so the PE clock gate
        # opens (K=8/8).  The per-step fillers keep it open afterwards.
        for i in range(40):
            nc.tensor.matmul(
                junk[:, :],
                lhsT=wh_ur_sb[:, 0, 0, 0:128],
                rhs=wh_ur_sb[:, 0, 0, 0:128],
                start=(i == 0),
                stop=(i == 39),
                skip_group_check=True,
            )

        def emit_x(t, pu_t, ph_t):
            """Input-projection matmuls for step t into fresh psum tiles.

            Broadcast-init (K=1 matmul of a zeros row) zeroes every
            partition incl. unused lanes, so downstream full-width reads
            are always on initialized data.
            """
            nc.tensor.matmul(
                pu_t[:, :],
                lhsT=ones_sb[:],
                rhs=zrow_sb[:, 0:256],
                start=True,
                stop=False,
                skip_group_check=True,
            )
            nc.tensor.matmul(
                ph_t[:, :],
                lhsT=ones_sb[:],
                rhs=zrow_sb[:, 256:384],
                start=True,
                stop=False,
                skip_group_check=True,
            )
            for kc in range(KC_X):
                st = xT_sb[:, kc, t, :]
                for g in range(4):
                    nc.tensor.matmul(
                        pu_t[32 * g : 32 * g + 16, 0:256],
                        lhsT=st,
                        rhs=wx_sb[:, kc, g, 0:256],
                        start=False,
                        stop=False,
                        tile_position=(0, 32 * g),
                        skip_group_check=True,
                    )
                    nc.tensor.matmul(
                        ph_t[32 * g : 32 * g + 16, :],
                        lhsT=st,
                        rhs=wx_sb[:, kc, g, 256:384],
                        start=False,
                        stop=False,
                        tile_position=(0, 32 * g),
                        skip_group_check=True,
                    )
            if with_bias:
                for g in range(4):
                    nc.tensor.matmul(
                        pu_t[32 * g : 32 * g + 16, 0:256],
                        lhsT=ones_sb[:, 0:16],
                        rhs=bias_sb[:, g, 0:256],
                        start=False,
                        stop=False,
                        tile_position=(0, 32 * g),
                        skip_group_check=True,
                    )
                    nc.tensor.matmul(
                        ph_t[32 * g : 32 * g + 16, :],
                        lhsT=ones_sb[:, 0:16],
                        rhs=bias_sb[:, g, 256:384],
                        start=False,
                        stop=False,
                        tile_position=(0, 32 * g),
                        skip_group_check=True,
                    )

        pu_cur = ps_ur.tile([128, 256], F32)
        ph_cur = ps_h.tile([128, 128], F32)
        emit_x(0, pu_cur, ph_cur)

        hT16_prev = h0_16
        h32_prev_ap = h0_32[:]
        stage_cur = None

        for t in range(t_steps):
            if t % OUT_BLOCK == 0:
                stage_cur = stage_pool.tile([128, OUT_BLOCK, 128], F32)
            h32_cur_ap = stage_cur[:, t % OUT_BLOCK, :]

            # ---- recurrent gate matmuls: r first (feeds the chain), then u
            for cols, c0 in ((slice(128, 256), 128), (slice(0, 128), 0)):
                for kc in range(KC_H):
                    for g in range(4):
                        nc.tensor.matmul(
                            pu_cur[32 * g : 32 * g + 16, cols],
                            lhsT=hT16_prev[:, 32 * kc : 32 * kc + 16],
                            rhs=wh_ur_sb[:, kc, g, c0 : c0 + 128],
                            start=False,
                            stop=(kc == KC_H - 1) and (g == 3),
                            tile_position=(0, 32 * g),
                            skip_group_check=True,
                        )

            # ---- x-projections for t+1: fill PE idle during sigmoid ----
            if t + 1 < t_steps:
                pu_nxt = ps_ur.tile([128, 256], F32)
                ph_nxt = ps_h.tile([128, 128], F32)
                emit_x(t + 1, pu_nxt, ph_nxt)
            else:
                pu_nxt = ph_nxt = None
            emit_fill(f_sig)

            # ---- ACT pacing dummy, then sigmoid(r) [chain], sigmoid(u) --
            if PACE_SIG_N > 0:
                nc.scalar.copy(
                    pace_dst[:, 0:PACE_SIG_N], pace_src[:, 0:PACE_SIG_N]
                )
            r_s = gates.tile([128, 128], BF16, tag="r_s")
            u_s = gates.tile([128, 128], F32, tag="u_s")
            nc.scalar.activation(
                r_s[:], pu_cur[:, 128:256], mybir.ActivationFunctionType.Sigmoid
            )
            nc.scalar.activation(
                u_s[:], pu_cur[:, 0:128], mybir.ActivationFunctionType.Sigmoid
            )

            # ---- transposes: rT (bf16, chain), uT (f32, off-chain) ----
            rT = ps_rT.tile([128, 128], BF16, tag="rT")
            nc.tensor.transpose(rT[:], r_s[:], ident16_sb[:])
            uT = ps_uT.tile([128, 128], F32, tag="uT")
            nc.tensor.transpose(uT[:], u_s[:], ident_sb[:])
            emit_fill(f_tt)

            # ---- tT = rT * hT (chain, DVE) ----
            tT = gates.tile([128, 128], BF16, tag="tT")
            nc.vector.tensor_mul(tT[:], rT[:], hT16_prev[:])

            # ---- uT evacuation PSUM->SBUF on ACT (off-chain) ----
            uT_sb = gates.tile([128, 128], F32, tag="uT_sb")
            nc.scalar.copy(uT_sb[:], uT[:])

            # ---- candidate matmuls (chain) ----
            for kc in range(KC_H):
                for g in range(4):
                    nc.tensor.matmul(
                        ph_cur[32 * g : 32 * g + 16, :],
                        lhsT=tT[:, 32 * kc : 32 * kc + 16],
                        rhs=wh_hh_sb[:, kc, g, :],
                        start=False,
                        stop=(kc == KC_H - 1) and (g == 3),
                        tile_position=(0, 32 * g),
                        skip_group_check=True,
                    )

            # ---- a-path (off-chain, DVE): a = h32 - uT*h32 ----
            a1 = gates.tile([128, 128], F32, tag="a1")
            nc.vector.tensor_mul(a1[:], uT_sb[:], h32_prev_ap)
            a32 = gates.tile([128, 128], F32, tag="a32")
            nc.vector.tensor_sub(a32[:], h32_prev_ap, a1[:])
            emit_fill(f_tanh)

            # ---- tanh (chain) ----
            hh_s = gates.tile([128, 128], F32, tag="hh_s")
            nc.scalar.activation(
                hh_s[:], ph_cur[:], mybir.ActivationFunctionType.Tanh
            )

            # ---- T_hh (chain, f32) ----
            hhT = ps_hhT.tile([128, 128], F32, tag="hhT")
            nc.tensor.transpose(hhT[:], hh_s[:], ident_sb[:])
            emit_fill(f_tail)

            # ---- tail: b = uT*hhT ; hT' = a + b (bf16 for MMs, f32 state) --
            b32 = gates.tile([128, 128], F32, tag="b32")
            nc.vector.tensor_mul(b32[:], uT_sb[:], hhT[:])
            hT16_new = hT_pool.tile([128, 128], BF16)
            nc.vector.tensor_add(hT16_new[:], a32[:], b32[:])
            nc.vector.tensor_add(h32_cur_ap, a32[:], b32[:])

            # ---- flush output block ----
            if (t + 1) % OUT_BLOCK == 0 or t == t_steps - 1:
                n = t % OUT_BLOCK + 1
                t0 = t - n + 1
                nc.sync.dma_start(
                    out=out[:, t0 : t0 + n, :],
                    in_=stage_cur[:, 0:n, :],
                )

            hT16_prev = hT16_new
            h32_prev_ap = h32_cur_ap
            pu_cur, ph_cur = pu_nxt, ph_nxt

    # Bacc.finalize runs the compile pipeline (wait splitting, register
    # allocation). run_bass_via_pjrt serializes nc as-is, so do it here.
    nc.finalize()
    return nc


# ---------------------------------------------------------------------------
# Host-side packing / unpacking
# ---------------------------------------------------------------------------


def _pack_x(xc):
    """xc [BL, t, 256] (already direction-adjusted) -> [128, 2, t, BL] bf16."""
    t = xc.shape[1]
    a = np.ascontiguousarray(xc.transpose(2, 1, 0))  # [256, t, BL]
    a = a.reshape(KC_X, 128, t, BL).transpose(1, 0, 2, 3)
    return np.ascontiguousarray(a).astype(bfloat16)


def _pack_wh(wh):
    u = wh[:, 0:512].reshape(512, 4, 128)
    r = wh[:, 512:1024].reshape(512, 4, 128)
    hh = wh[:, 1024:1536].reshape(512, 4, 128)
    ur = np.concatenate([u, r], axis=2)  # [512, 4, 256]
    ur = ur.reshape(KC_H, 128, 4, 256).transpose(1, 0, 2, 3)
    hh = hh.reshape(KC_H, 128, 4, 128).transpose(1, 0, 2, 3)
    return (
        np.ascontiguousarray(ur).astype(bfloat16),
        np.ascontiguousarray(hh).astype(bfloat16),
    )


def _pack_wx(wx):
    u = wx[:, 0:512].reshape(256, 4, 128)
    r = wx[:, 512:1024].reshape(256, 4, 128)
    hh = wx[:, 1024:1536].reshape(256, 4, 128)
    a = np.concatenate([u, r, hh], axis=2)  # [256, 4, 384]
    a = a.reshape(KC_X, 128, 4, 384).transpose(1, 0, 2, 3)
    return np.ascontiguousarray(a).astype(bfloat16)


def _pack_bias(b):
    u = b[0:512].reshape(4, 128)
    r = b[512:1024].reshape(4, 128)
    hh = b[1024:1536].reshape(4, 128)
    a = np.concatenate([u, r, hh], axis=1)[None]  # [1, 4, 384]
    return np.ascontiguousarray(a.astype(bfloat16))


def make_in_maps(x, Wx_f, Wh_f, b_f, Wx_b, Wh_b, b_b, t_steps=T, with_bias=False):
    x = np.asarray(x, dtype=np.float32)
    ident = np.eye(128, dtype=np.float32)
    per_dir = {}
    for d, (wx, wh, bb) in enumerate(
        [(Wx_f, Wh_f, b_f), (Wx_b, Wh_b, b_b)]
    ):
        ur, hh = _pack_wh(np.asarray(wh, np.float32))
        wxp = _pack_wx(np.asarray(wx, np.float32))
        ent = {"wh_ur": ur, "wh_hh": hh, "wx_all": wxp}
        if with_bias:
            ent["bias_all"] = _pack_bias(np.asarray(bb, np.float32))
        per_dir[d] = ent

    in_maps = []
    for c in range(NCORES):
        d, q = divmod(c, 4)
        xc = x[q * BL : (q + 1) * BL, :t_steps]
        if d == 1:
            xc = xc[:, ::-1]
        m = {
            "xT": _pack_x(xc),
            "ident": ident,
            "ident16": ident.astype(bfloat16),
            "zrow": np.zeros((1, 384), dtype=bfloat16),
            "ones_row": np.ones((1, 128), dtype=bfloat16),
        }
        m.update(per_dir[d])
        in_maps.append(m)
    return in_maps


def unpack_outputs(results, t_steps=T):
    out = np.empty((B, t_steps, 2 * U), dtype=np.float32)
    for c in range(NCORES):
        d, q = divmod(c, 4)
        # packed [c, t, 32g+b] -> out[b, t, 128g+c]
        r = results[c]["out_packed"].reshape(128, t_steps, 4, 32)[:, :, :, 0:BL]
        out[q * BL : (q + 1) * BL, :, d * U : (d + 1) * U] = (
            r.transpose(3, 1, 2, 0).reshape(BL, t_steps, U)
        )
    return out


_CACHE = {}


def kernel(x, Wx_f, Wh_f, b_f, Wx_b, Wh_b, b_b):
    with_bias = bool(
        np.any(np.asarray(b_f) != 0) or np.any(np.asarray(b_b) != 0)
    )
    key = ("prog", T, with_bias)
    if key not in _CACHE:
        _CACHE[key] = build_program(T, with_bias)
    nc = _CACHE[key]
    in_maps = make_in_maps(
        x, Wx_f, Wh_f, b_f, Wx_b, Wh_b, b_b, T, with_bias
    )
    res = run_bass_kernel_spmd(nc, in_maps, list(range(NCORES)))
    return unpack_outputs(res.results, T)


if __name__ == "__main__":
    mode = sys.argv[1] if len(sys.argv) > 1 else "sim"
    if mode == "sim":
        # Small-T single-core simulation vs numpy GRU.
        ts = int(sys.argv[2]) if len(sys.argv) > 2 else 8
        rng = np.random.default_rng(0)
        x = rng.standard_normal((B, ts, D), dtype=np.float32)
        Wx = (rng.standard_normal((D, 3 * U), dtype=np.float32) / np.sqrt(D)).astype(
            np.float32
        )
        Wh = (rng.standard_normal((U, 3 * U), dtype=np.float32) / np.sqrt(U)).astype(
            np.float32
        )
        bz = np.zeros(3 * U, np.float32)

        nc = build_program(ts, with_bias=False)
        in_maps = make_in_maps(x, Wx, Wh, bz, Wx, Wh, bz, ts, False)

        from concourse.bass_interp import MultiCoreSim

        sim = MultiCoreSim(nc, 1)
        for k, v in in_maps[0].items():
            sim.cores[0].tensor(k)[:] = v
        sim.simulate()
        got = sim.cores[0].tensor("out_packed")  # [128, ts, 128]
        got = (
            got.reshape(128, ts, 4, 32)[:, :, :, 0:BL]
            .transpose(3, 1, 2, 0)
            .reshape(BL, ts, U)
        )

        # numpy reference GRU (forward, batch quarter 0)
        h = np.zeros((BL, U), np.float32)
        exp = np.zeros((BL, ts, U), np.float32)
        xs = x[0:BL, :ts].astype(np.float32)
        for t in range(ts):
            gx = xs[:, t] @ Wx
            pu = gx[:, :U] + h @ Wh[:, :U]
            pr = gx[:, U : 2 * U] + h @ Wh[:, U : 2 * U]
            u = 1 / (1 + np.exp(-pu))
            r = 1 / (1 + np.exp(-pr))
            hh = np.tanh(gx[:, 2 * U :] + (r * h) @ Wh[:, 2 * U :])
            h = (1 - u) * h + u * hh
            exp[:, t] = h
        err = np.abs(got - exp)
        denom = max(1e-6, np.abs(exp).max())
        print("max abs err:", err.max(), " rel:", err.max() / denom)
        print("sample got:", got[0, -1, :4], " exp:", exp[0, -1, :4])
    else:
        print("unknown mode", mode)


# revision 11
# speedup vs baseline: 1.2550x; 1.2550x over previous
"""BiGRU Trainium2 kernel: B=64, T=512, D=256, U=512, 8 NeuronCores.

Sharding: 8 cores = 2 directions x 4 batch-quarters (B_local=16).
The GRU recurrence is latency-bound (T sequential steps); each core runs one
chain for (direction, batch-quarter) with a packed layout:
  partition p = 32*g + b   (g = U-block 0..3, b = local batch 0..15)
  column   c = offset within U-block (0..127);  u = 128*g + c

Gate matmuls are column-tiled across the 4 PE column groups (concurrent on
HW).  The state is kept TRANSPOSED only: hT32 [c, (g,b)] in fp32 (exact
accumulation; also the DMA'd output) plus a bf16 copy hT16 for the matmul
stationary operand.  Update:  hT_new = a + uT*hhT,  a = hT32 - uT*hT32
(a computed off the critical chain).  Filler matmuls into a junk PSUM bank
keep the PE HAM clock gate open (2.4 GHz) through the serial-chain gaps.
"""

import sys
import os

for _p in ("/opt/trn_rl_repo",):
    if os.path.isdir(_p) and _p not in sys.path:
        sys.path.insert(0, _p)

import numpy as np
from contextlib import ExitStack

import concourse.bass as bass
import concourse.bacc as bacc
import concourse.tile as tile
from concourse import mybir
from concourse.bass_utils import run_bass_kernel_spmd

try:
    from ml_dtypes import bfloat16
except ImportError:  # pragma: no cover
    import jax.numpy as _jnp

    bfloat16 = _jnp.bfloat16

B, T, D, U = 64, 512, 256, 512
NCORES = 8
BL = B // 4  # 16 local batch per core (4 batch quarters x 2 directions)
NG = 4  # U blocks of 128
KC_H = 4  # contraction chunks over U (512/128)
KC_X = 2  # contraction chunks over D (256/128)

F32 = mybir.dt.float32
BF16 = mybir.dt.bfloat16

OUT_BLOCK = 8  # steps per output DMA flush

# PE warm-keeping filler counts (matmuls into a junk PSUM bank placed in the
# PE stream where the serial chain would otherwise leave the array idle long
# enough for the HAM clock gate to re-throttle to 1.2 GHz).
FILL_SIG = 10   # during sigmoid(r) latency
FILL_TT = 4     # during tT = rT*hT
FILL_TANH = 6   # during tanh latency (T_u now pops here too)
FILL_TAIL = 10  # during b/hT_new tail


def build_program(t_steps=T, with_bias=False, fills=None):
    """Builds the SPMD Bass program (identical for all cores)."""
    f_sig, f_tt, f_tanh, f_tail = fills or (FILL_SIG, FILL_TT, FILL_TANH, FILL_TAIL)
    # Bacc (not plain Bass): its compile pipeline splits multi-sem waits into
    # EventSemaphore instructions — TRN2 instructions hold at most one wait.
    nc = bacc.Bacc(None, target_bir_lowering=False)

    xT = nc.dram_tensor("xT", [128, KC_X, t_steps, BL], BF16, kind="ExternalInput")
    wh_ur = nc.dram_tensor("wh_ur", [128, KC_H, NG, 256], BF16, kind="ExternalInput")
    wh_hh = nc.dram_tensor("wh_hh", [128, KC_H, NG, 128], BF16, kind="ExternalInput")
    wx_all = nc.dram_tensor("wx_all", [128, KC_X, NG, 384], BF16, kind="ExternalInput")
    ident = nc.dram_tensor("ident", [128, 128], F32, kind="ExternalInput")
    ident16 = nc.dram_tensor("ident16", [128, 128], BF16, kind="ExternalInput")
    zrow = nc.dram_tensor("zrow", [1, 384], BF16, kind="ExternalInput")
    ones_row = nc.dram_tensor("ones_row", [1, 128], BF16, kind="ExternalInput")
    if with_bias:
        bias_all = nc.dram_tensor("bias_all", [1, NG, 384], BF16, kind="ExternalInput")
    out = nc.dram_tensor("out_packed", [128, t_steps, 128], F32, kind="ExternalOutput")

    with tile.TileContext(nc) as tc, ExitStack() as ctx:
        singles = ctx.enter_context(tc.tile_pool(name="singles", bufs=1))
        hT_pool = ctx.enter_context(tc.tile_pool(name="hT", bufs=3))
        gates = ctx.enter_context(tc.tile_pool(name="gates", bufs=3))
        stage_pool = ctx.enter_context(tc.tile_pool(name="stage", bufs=2))
        # bufs=1: the cur/next allocation pattern already makes 2 live slots
        # per pool (PSUM slots are bank-rounded; 8 banks total).
        ps_ur = ctx.enter_context(tc.tile_pool(name="ps_ur", bufs=1, space="PSUM"))
        ps_h = ctx.enter_context(tc.tile_pool(name="ps_h", bufs=1, space="PSUM"))
        ps_rT = ctx.enter_context(tc.tile_pool(name="ps_rT", bufs=1, space="PSUM"))
        ps_uT = ctx.enter_context(tc.tile_pool(name="ps_uT", bufs=1, space="PSUM"))
        ps_hhT = ctx.enter_context(tc.tile_pool(name="ps_hhT", bufs=1, space="PSUM"))
        ps_junk = ctx.enter_context(tc.tile_pool(name="ps_junk", bufs=1, space="PSUM"))

        # --- resident inputs ---
        xT_sb = singles.tile([128, KC_X, t_steps, BL], BF16)
        wh_ur_sb = singles.tile([128, KC_H, NG, 256], BF16)
        wh_hh_sb = singles.tile([128, KC_H, NG, 128], BF16)
        wx_sb = singles.tile([128, KC_X, NG, 384], BF16)
        ident_sb = singles.tile([128, 128], F32)
        ident16_sb = singles.tile([128, 128], BF16)
        nc.sync.dma_start(out=ident16_sb[:], in_=ident16[:])
        nc.sync.dma_start(out=xT_sb[:], in_=xT[:])
        nc.sync.dma_start(out=wh_ur_sb[:], in_=wh_ur[:])
        nc.sync.dma_start(out=wh_hh_sb[:], in_=wh_hh[:])
        nc.sync.dma_start(out=wx_sb[:], in_=wx_all[:])
        nc.sync.dma_start(out=ident_sb[:], in_=ident[:])
        zrow_sb = singles.tile([1, 384], BF16)
        ones_sb = singles.tile([1, 128], BF16)
        nc.sync.dma_start(out=zrow_sb[:], in_=zrow[:])
        nc.sync.dma_start(out=ones_sb[:], in_=ones_row[:])
        if with_bias:
            bias_sb = singles.tile([1, NG, 384], BF16)
            nc.sync.dma_start(out=bias_sb[:], in_=bias_all[:])

        # --- initial state (h = 0) ---
        h0_16 = singles.tile([128, 128], BF16)
        h0_32 = singles.tile([128, 128], F32)
        nc.vector.memset(h0_16[:], 0.0)
        nc.vector.memset(h0_32[:], 0.0)

        junk = ps_junk.tile([128, 128], F32, tag="junk")

        def emit_fill(n):
            for _ in range(n):
                nc.tensor.matmul(
                    junk[0:16, :],
                    lhsT=wh_ur_sb[:, 0, 0, 0:16],
                    rhs=wh_ur_sb[:, 0, 0, 0:128],
                    start=True,
                    stop=True,
                    tile_position=(0, 0),
                    skip_group_check=True,
                )

        # Pre-consume idents on PE: transpose-mode matmuls lower to the LDW
        # struct which holds only ONE sync wait; without this the first real
        # transpose would need both its data wait and the ident-DMA wait.
        warm_rT = ps_rT.tile([128, 128], BF16, tag="rT")
        nc.tensor.transpose(warm_rT[:], ident16_sb[:], ident16_sb[:])
        warm_uT = ps_uT.tile([128, 128], F32, tag="uT")
        nc.tensor.transpose(warm_uT[:], ident_sb[:], ident_sb[:])

        # HAM warm-up: ~5us of back-to-back matmuls so the PE clock gate
        # opens (K=8/8).  The per-step fillers keep it open afterwards.
        for i in range(40):
            nc.tensor.matmul(
                junk[:, :],
                lhsT=wh_ur_sb[:, 0, 0, 0:128],
                rhs=wh_ur_sb[:, 0, 0, 0:128],
                start=(i == 0),
                stop=(i == 39),
                skip_group_check=True,
            )

        def emit_x(t, pu_t, ph_t):
            """Input-projection matmuls for step t into fresh psum tiles.

            Broadcast-init (K=1 matmul of a zeros row) zeroes every
            partition incl. unused lanes, so downstream full-width reads
            are always on initialized data.
            """
            nc.tensor.matmul(
                pu_t[:, :],
                lhsT=ones_sb[:],
                rhs=zrow_sb[:, 0:256],
                start=True,
                stop=False,
                skip_group_check=True,
            )
            nc.tensor.matmul(
                ph_t[:, :],
                lhsT=ones_sb[:],
                rhs=zrow_sb[:, 256:384],
                start=True,
                stop=False,
                skip_group_check=True,
            )
            for kc in range(KC_X):
                st = xT_sb[:, kc, t, :]
                for g in range(4):
                    nc.tensor.matmul(
                        pu_t[32 * g : 32 * g + 16, 0:256],
                        lhsT=st,
                        rhs=wx_sb[:, kc, g, 0:256],
                        start=False,
                        stop=False,
                        tile_position=(0, 32 * g),
                        skip_group_check=True,
                    )
                    nc.tensor.matmul(
                        ph_t[32 * g : 32 * g + 16, :],
                        lhsT=st,
                        rhs=wx_sb[:, kc, g, 256:384],
                        start=False,
                        stop=False,
                        tile_position=(0, 32 * g),
                        skip_group_check=True,
                    )
            if with_bias:
                for g in range(4):
                    nc.tensor.matmul(
                        pu_t[32 * g : 32 * g + 16, 0:256],
                        lhsT=ones_sb[:, 0:16],
                        rhs=bias_sb[:, g, 0:256],
                        start=False,
                        stop=False,
                        tile_position=(0, 32 * g),
                        skip_group_check=True,
                    )
                    nc.tensor.matmul(
                        ph_t[32 * g : 32 * g + 16, :],
                        lhsT=ones_sb[:, 0:16],
                        rhs=bias_sb[:, g, 256:384],
                        start=False,
                        stop=False,
                        tile_position=(0, 32 * g),
                        skip_group_check=True,
                    )

        pu_cur = ps_ur.tile([128, 256], F32)
        ph_cur = ps_h.tile([128, 128], F32)
        emit_x(0, pu_cur, ph_cur)

        hT16_prev = h0_16
        h32_prev_ap = h0_32[:]
        stage_cur = None

        for t in range(t_steps):
            if t % OUT_BLOCK == 0:
                stage_cur = stage_pool.tile([128, OUT_BLOCK, 128], F32)
            h32_cur_ap = stage_cur[:, t % OUT_BLOCK, :]

            # ---- recurrent gate matmuls: r first (feeds the chain), then u
            for cols, c0 in ((slice(128, 256), 128), (slice(0, 128), 0)):
                for kc in range(KC_H):
                    for g in range(4):
                        nc.tensor.matmul(
                            pu_cur[32 * g : 32 * g + 16, cols],
                            lhsT=hT16_prev[:, 32 * kc : 32 * kc + 16],
                            rhs=wh_ur_sb[:, kc, g, c0 : c0 + 128],
                            start=False,
                            stop=(kc == KC_H - 1) and (g == 3),
                            tile_position=(0, 32 * g),
                            skip_group_check=True,
                        )

            # ---- x-projections for t+1: fill PE idle during sigmoid ----
            if t + 1 < t_steps:
                pu_nxt = ps_ur.tile([128, 256], F32)
                ph_nxt = ps_h.tile([128, 128], F32)
                emit_x(t + 1, pu_nxt, ph_nxt)
            else:
                pu_nxt = ph_nxt = None
            emit_fill(f_sig)

            # ---- sigmoid(r) [chain], sigmoid(u) ----
            r_s = gates.tile([128, 128], BF16, tag="r_s")
            u_s = gates.tile([128, 128], F32, tag="u_s")
            nc.scalar.activation(
                r_s[:], pu_cur[:, 128:256], mybir.ActivationFunctionType.Sigmoid
            )
            nc.scalar.activation(
                u_s[:], pu_cur[:, 0:128], mybir.ActivationFunctionType.Sigmoid
            )

            # ---- transposes: rT (bf16, chain), uT (f32, off-chain) ----
            rT = ps_rT.tile([128, 128], BF16, tag="rT")
            nc.tensor.transpose(rT[:], r_s[:], ident16_sb[:])
            emit_fill(f_tt)

            # ---- tT = rT * hT (chain, DVE) ----
            tT = gates.tile([128, 128], BF16, tag="tT")
            nc.vector.tensor_mul(tT[:], rT[:], hT16_prev[:])

            # ---- candidate matmuls (chain) ----
            for kc in range(KC_H):
                for g in range(4):
                    nc.tensor.matmul(
                        ph_cur[32 * g : 32 * g + 16, :],
                        lhsT=tT[:, 32 * kc : 32 * kc + 16],
                        rhs=wh_hh_sb[:, kc, g, :],
                        start=False,
                        stop=(kc == KC_H - 1) and (g == 3),
                        tile_position=(0, 32 * g),
                        skip_group_check=True,
                    )

            # ---- T_u moved here: pops after the hh matmuls, filling the
            # tanh window instead of jamming the tT->hh handoff ----
            uT = ps_uT.tile([128, 128], F32, tag="uT")
            nc.tensor.transpose(uT[:], u_s[:], ident_sb[:])
            emit_fill(f_tanh)

            # ---- a-path (off-chain, DVE): q = (uT - 1)*h32 = -a ----
            q32 = gates.tile([128, 128], F32, tag="q32")
            nc.vector.scalar_tensor_tensor(
                q32[:], uT[:], 1.0, h32_prev_ap,
                mybir.AluOpType.subtract, mybir.AluOpType.mult,
            )

            # ---- tanh (chain) ----
            hh_s = gates.tile([128, 128], F32, tag="hh_s")
            nc.scalar.activation(
                hh_s[:], ph_cur[:], mybir.ActivationFunctionType.Tanh
            )
            uT_sb = gates.tile([128, 128], F32, tag="uT_sb")
            nc.scalar.copy(uT_sb[:], uT[:])

            # ---- T_hh (chain, f32) ----
            hhT = ps_hhT.tile([128, 128], F32, tag="hhT")
            nc.tensor.transpose(hhT[:], hh_s[:], ident_sb[:])
            emit_fill(f_tail)

            # ---- tail: b = uT*hhT ; hT' = a + b (bf16 for MMs, f32 state) --
            b32 = gates.tile([128, 128], F32, tag="b32")
            nc.vector.tensor_mul(b32[:], uT_sb[:], hhT[:])
            hT16_new = hT_pool.tile([128, 128], BF16)
            nc.vector.tensor_sub(hT16_new[:], b32[:], q32[:])
            nc.vector.tensor_sub(h32_cur_ap, b32[:], q32[:])

            # ---- flush output block ----
            if (t + 1) % OUT_BLOCK == 0 or t == t_steps - 1:
                n = t % OUT_BLOCK + 1
                t0 = t - n + 1
                nc.sync.dma_start(
                    out=out[:, t0 : t0 + n, :],
                    in_=stage_cur[:, 0:n, :],
                )

            hT16_prev = hT16_new
            h32_prev_ap = h32_cur_ap
            pu_cur, ph_cur = pu_nxt, ph_nxt

    # Bacc.finalize runs the compile pipeline (wait splitting, register
    # allocation). run_bass_via_pjrt serializes nc as-is, so do it here.
    nc.finalize()
    return nc


# ---------------------------------------------------------------------------
# Host-side packing / unpacking
# ---------------------------------------------------------------------------


def _pack_x(xc):
    """xc [BL, t, 256] (already direction-adjusted) -> [128, 2, t, BL] bf16."""
    t = xc.shape[1]
    a = np.ascontiguousarray(xc.transpose(2, 1, 0))  # [256, t, BL]
    a = a.reshape(KC_X, 128, t, BL).transpose(1, 0, 2, 3)
    return np.ascontiguousarray(a).astype(bfloat16)


def _pack_wh(wh):
    u = wh[:, 0:512].reshape(512, 4, 128)
    r = wh[:, 512:1024].reshape(512, 4, 128)
    hh = wh[:, 1024:1536].reshape(512, 4, 128)
    ur = np.concatenate([u, r], axis=2)  # [512, 4, 256]
    ur = ur.reshape(KC_H, 128, 4, 256).transpose(1, 0, 2, 3)
    hh = hh.reshape(KC_H, 128, 4, 128).transpose(1, 0, 2, 3)
    return (
        np.ascontiguousarray(ur).astype(bfloat16),
        np.ascontiguousarray(hh).astype(bfloat16),
    )


def _pack_wx(wx):
    u = wx[:, 0:512].reshape(256, 4, 128)
    r = wx[:, 512:1024].reshape(256, 4, 128)
    hh = wx[:, 1024:1536].reshape(256, 4, 128)
    a = np.concatenate([u, r, hh], axis=2)  # [256, 4, 384]
    a = a.reshape(KC_X, 128, 4, 384).transpose(1, 0, 2, 3)
    return np.ascontiguousarray(a).astype(bfloat16)


def _pack_bias(b):
    u = b[0:512].reshape(4, 128)
    r = b[512:1024].reshape(4, 128)
    hh = b[1024:1536].reshape(4, 128)
    a = np.concatenate([u, r, hh], axis=1)[None]  # [1, 4, 384]
    return np.ascontiguousarray(a.astype(bfloat16))


def make_in_maps(x, Wx_f, Wh_f, b_f, Wx_b, Wh_b, b_b, t_steps=T, with_bias=False):
    x = np.asarray(x, dtype=np.float32)
    ident = np.eye(128, dtype=np.float32)
    per_dir = {}
    for d, (wx, wh, bb) in enumerate(
        [(Wx_f, Wh_f, b_f), (Wx_b, Wh_b, b_b)]
    ):
        ur, hh = _pack_wh(np.asarray(wh, np.float32))
        wxp = _pack_wx(np.asarray(wx, np.float32))
        ent = {"wh_ur": ur, "wh_hh": hh, "wx_all": wxp}
        if with_bias:
            ent["bias_all"] = _pack_bias(np.asarray(bb, np.float32))
        per_dir[d] = ent

    in_maps = []
    for c in range(NCORES):
        d, q = divmod(c, 4)
        xc = x[q * BL : (q + 1) * BL, :t_steps]
        if d == 1:
            xc = xc[:, ::-1]
        m = {
            "xT": _pack_x(xc),
            "ident": ident,
            "ident16": ident.astype(bfloat16),
            "zrow": np.zeros((1, 384), dtype=bfloat16),
            "ones_row": np.ones((1, 128), dtype=bfloat16),
        }
        m.update(per_dir[d])
        in_maps.append(m)
    return in_maps


def unpack_outputs(results, t_steps=T):
    out = np.empty((B, t_steps, 2 * U), dtype=np.float32)
    for c in range(NCORES):
        d, q = divmod(c, 4)
        # packed [c, t, 32g+b] -> out[b, t, 128g+c]
        r = results[c]["out_packed"].reshape(128, t_steps, 4, 32)[:, :, :, 0:BL]
        out[q * BL : (q + 1) * BL, :, d * U : (d + 1) * U] = (
            r.transpose(3, 1, 2, 0).reshape(BL, t_steps, U)
        )
    return out


_CACHE = {}


def kernel(x, Wx_f, Wh_f, b_f, Wx_b, Wh_b, b_b):
    with_bias = bool(
        np.any(np.asarray(b_f) != 0) or np.any(np.asarray(b_b) != 0)
    )
    key = ("prog", T, with_bias)
    if key not in _CACHE:
        _CACHE[key] = build_program(T, with_bias)
    nc = _CACHE[key]
    in_maps = make_in_maps(
        x, Wx_f, Wh_f, b_f, Wx_b, Wh_b, b_b, T, with_bias
    )
    res = run_bass_kernel_spmd(nc, in_maps, list(range(NCORES)))
    return unpack_outputs(res.results, T)


if __name__ == "__main__":
    mode = sys.argv[1] if len(sys.argv) > 1 else "sim"
    if mode == "sim":
        # Small-T single-core simulation vs numpy GRU.
        ts = int(sys.argv[2]) if len(sys.argv) > 2 else 8
        rng = np.random.default_rng(0)
        x = rng.standard_normal((B, ts, D), dtype=np.float32)
        Wx = (rng.standard_normal((D, 3 * U), dtype=np.float32) / np.sqrt(D)).astype(
            np.float32
        )
        Wh = (rng.standard_normal((U, 3 * U), dtype=np.float32) / np.sqrt(U)).astype(
            np.float32
        )
        bz = np.zeros(3 * U, np.float32)

        nc = build_program(ts, with_bias=False)
        in_maps = make_in_maps(x, Wx, Wh, bz, Wx, Wh, bz, ts, False)

        from concourse.bass_interp import MultiCoreSim

        sim = MultiCoreSim(nc, 1)
        for k, v in in_maps[0].items():
            sim.cores[0].tensor(k)[:] = v
        sim.simulate()
        got = sim.cores[0].tensor("out_packed")  # [128, ts, 128]
        got = (
            got.reshape(128, ts, 4, 32)[:, :, :, 0:BL]
            .transpose(3, 1, 2, 0)
            .reshape(BL, ts, U)
        )

        # numpy reference GRU (forward, batch quarter 0)
        h = np.zeros((BL, U), np.float32)
        exp = np.zeros((BL, ts, U), np.float32)
        xs = x[0:BL, :ts].astype(np.float32)
        for t in range(ts):
            gx = xs[:, t] @ Wx
            pu = gx[:, :U] + h @ Wh[:, :U]
            pr = gx[:, U : 2 * U] + h @ Wh[:, U : 2 * U]
            u = 1 / (1 + np.exp(-pu))
            r = 1 / (1 + np.exp(-pr))
            hh = np.tanh(gx[:, 2 * U :] + (r * h) @ Wh[:, 2 * U :])
            h = (1 - u) * h + u * hh
            exp[:, t] = h
        err = np.abs(got - exp)
        denom = max(1e-6, np.abs(exp).max())
        print("max abs err:", err.max(), " rel:", err.max() / denom)
        print("sample got:", got[0, -1, :4], " exp:", exp[0, -1, :4])
    else:
        print("unknown mode", mode)
